# revision 11
# baseline (speedup 1.0000x reference)
"""AutoCorrelation (factor=3) Trainium2 kernel, 8 NeuronCores, batch-parallel.

Math. The reference computes corr = irfft(rfft(q, L) * conj(rfft(k, L)),
2047) over the padded feature axis, but only ever uses mean_l corr --
which collapses to quadratic forms of the Gram matrix N = k^T q:
    Zbar[f] = sum_{d1,d2} N[d2,d1] e^{-i 2pi f (d1-d2)/L}
            = sum_Delta G[Delta] e^{-i 2pi f Delta/L},
where G[Delta] is the sum of the Delta-th diagonal of N. The final
weighted roll-sum is a circulant matmul out[l] = sum_m At[m,l] v[m],
At[m,l] = coef[(m-l) mod L], coef = scatter of the 20 softmax weights.

Device work (per core b = batch b, pure data parallel, no collectives):
  NEFF1: N = k^T q (32 fp32r matmuls; q,k stream in as [128, 4096]
    row-grouped views so each partition's DMA run is 16KB, the l-order
    of the contraction being free). N bounces through a zero-padless
    DRAM buffer and comes back as 4 combined skewed windows [128,1024]
    (partition stride 1537) whose column c holds diagonal Delta = c-512;
    gpsimd affine_selects zero the out-of-triangle garbage and a
    ones-vector matmul on the otherwise-idle PE column-sums the four
    windows straight into G [1024] (4KB shipped to host, vs 2MB before).
  NEFF2: out = At-circulant @ v. At is BLOCK-circulant: block (mt,lt)
    depends only on (mt-lt) mod 8, so only 8 distinct 128x128 blocks
    D_j[k,m] = coef[(128j+k-m) mod 1024] exist (512KB loaded, vs the 4MB
    dense At). j-outer loop keeps each stationary D_j on the PE for 8
    matmuls with all 8 PSUM banks accumulating; the output leaves as one
    [128, 4096] partition-major buffer (host un-permutes for free).
Host between launches: mean_value = G @ KER (KER folds the Delta-DFT
and the irfft-to-2047); top-20 + softmax; batch-0 shifts broadcast.

fp32r: IEEE fp32 bits processed by the PE at 1 cycle/row (4x fp32) with
~19-bit effective mantissa; rel err ~2e-4 vs the f64 oracle, and the
top-k selection margins (2e-3..1e-2 rel) keep the reference selection.
"""
import math
import numpy as np

from contextlib import ExitStack
from concourse import bass, mybir, tile, bacc
from concourse.bass_utils import run_bass_kernel_spmd

B, L, D = 8, 1024, 512
NF = L // 2 + 1      # 513
T = 2 * L - 1        # 2047
K = int(3 * math.log(float(L)))  # 20
F32 = mybir.dt.float32

# matmul compute dtype: float32 (safe) or float32r (full-rate fp32 path)
MM_DT = mybir.dt.float32r

NCORES = 8
CORE_IDS = list(range(NCORES))

_cache = {}


# ---------------------------------------------------------------- tables
def _tables():
    """KER[j, t]: mean_value = G @ KER, where G[j] is the diagonal sum of
    N = k^T q at offset Delta = j - 512. Combines the d-axis DFT of G with
    the irfft-to-2047 of Zbar/L (both tiny, fused into one [1024, 2047]
    host matrix)."""
    if 'tables' in _cache:
        return _cache['tables']
    f = np.arange(NF)

    ang2 = 2 * np.pi * np.outer(f, np.arange(T)) / T   # [513, 2047]
    alpha = np.full(NF, 2.0); alpha[0] = 1.0
    C2 = alpha[:, None] * np.cos(ang2) / (T * L)
    S2 = -2.0 * np.sin(ang2) / (T * L); S2[0] = 0.0

    delta = np.arange(1024) - 512                      # [1024]
    angd = 2 * np.pi * np.outer(delta, f) / L          # [1024, 513]
    KER = np.cos(angd) @ C2 - np.sin(angd) @ S2        # [1024, 2047]

    tabs = dict(KER=np.ascontiguousarray(KER, np.float32))
    _cache['tables'] = tabs
    return tabs


# ---------------------------------------------------------------- NEFF 1
def build_neff1():
    """N = k^T q on the PE; bounce N rows through a padded DRAM buffer
    (rows 1536 wide, data in cols [512,1024)); re-read 4 combined skewed
    windows [128,1024] at partition stride 1537 so window column c holds
    diagonal Delta = c - 512 for every row; affine_select masks the
    static invalid triangles (512 <= R+c < 1024 is the valid band); a
    ones-lhsT matmul partition-sums all 4 windows into G [2, 512]."""
    nc = bacc.Bacc(None, target_bir_lowering=False, debug=False)
    q_d = nc.declare_dram_parameter('q', [L, D], MM_DT, isOutput=False)
    k_d = nc.declare_dram_parameter('k', [L, D], MM_DT, isOutput=False)
    z_d = nc.declare_dram_parameter('zout', [1, 1024], F32, isOutput=True)

    LT, DT = L // 128, D // 128        # 8, 4
    ROWW = 3 * 512                     # padded row width in the bounce buf
    SKEW = ROWW + 1

    with tile.TileContext(nc) as tc, ExitStack() as ctx:
        pool = ctx.enter_context(tc.tile_pool(name='sb', bufs=1))
        skp = ctx.enter_context(tc.tile_pool(name='sk', bufs=4))
        psum = ctx.enter_context(
            tc.tile_pool(name='ps', bufs=1, space=bass.MemorySpace.PSUM))
        dram = ctx.enter_context(tc.tile_pool(name='dr', bufs=1, space='DRAM'))

        # flat bounce buffer; extra tail so the [128,1537] windows exist
        n2f = dram.tile([D * ROWW + 2048], MM_DT)

        def rows(t, w=ROWW):
            # [128, w]-strided view of row block t of the bounce buffer
            return n2f[t * 128 * w: (t + 1) * 128 * w].rearrange(
                '(p c) -> p c', c=w)

        def window(t):
            # combined skewed window: [p, c] = bounce[row R=128t+p,
            # col R + c]; data cols are [512, 1024) so col c holds
            # N[R, R + c - 512], i.e. diagonal Delta = c - 512.
            start = t * 128 * SKEW
            return n2f[start: start + 128 * SKEW].rearrange(
                '(p c) -> p c', c=SKEW)[:, 0:1024]

        # per-chunk input tiles: matmuls gate on single 256KB chunks, not
        # the whole 4MB (tile-granularity deps made the PE start ~16us in
        # when q/k were single [128, 8, 512] tiles)
        q_ts, k_ts = [], []
        for j in range(LT):
            q_t = pool.tile([128, D], MM_DT, tag=f'q{j}', name=f'q{j}')
            k_t = pool.tile([128, D], MM_DT, tag=f'k{j}', name=f'k{j}')
            nc.sync.dma_start(q_t[:], q_d[j * 128:(j + 1) * 128, :])
            nc.scalar.dma_start(k_t[:], k_d[j * 128:(j + 1) * 128, :])
            q_ts.append(q_t); k_ts.append(k_t)

        ones_f = pool.tile([128, 1], F32)
        nc.vector.memset(ones_f[:], 1.0)
        ones = pool.tile([128, 1], MM_DT)
        nc.vector.tensor_copy(ones[:], ones_f[:])

        # N[d2, d1] = sum_l k[l,d2] q[l,d1]; t2-OUTER so PSUM bank t2
        # completes after its 8 matmuls and the bounce/skew/mask/colsum
        # pipeline for row block t2 overlaps the Gram work of t2+1.
        pns = [psum.tile([128, D], F32, tag=f'pn{t2}', name=f'pn{t2}')
               for t2 in range(DT)]
        gps = [psum.tile([1, 512], F32, tag=f'g{h}', name=f'g{h}')
               for h in range(2)]
        wins = []
        for t2 in range(DT):
            for j in range(LT):
                nc.tensor.matmul(
                    pns[t2][:],
                    k_ts[j][:, t2 * 128:(t2 + 1) * 128],
                    q_ts[j][:],
                    start=(j == 0), stop=(j == LT - 1))
            n_t = skp.tile([128, 512], MM_DT, tag=f'nt{t2}', name=f'nt{t2}')
            nc.vector.tensor_copy(n_t[:], pns[t2][:])
            eng = nc.sync if t2 % 2 == 0 else nc.scalar
            eng.dma_start(rows(t2)[:, 512:1024], n_t[:])
            w_t = skp.tile([128, 1024], MM_DT, tag=f'w{t2}', name=f'w{t2}')
            eng.dma_start(w_t[:], window(t2))
            # mask the static invalid triangles: window t element [p, c]
            # is valid iff 512 <= (128t + p) + c < 1024
            nc.gpsimd.affine_select(
                w_t[:], w_t[:], pattern=[[1, 1024]],
                compare_op=mybir.AluOpType.is_ge, fill=0.0,
                base=128 * t2 - 512, channel_multiplier=1)
            nc.gpsimd.affine_select(
                w_t[:], w_t[:], pattern=[[-1, 1024]],
                compare_op=mybir.AluOpType.is_ge, fill=0.0,
                base=1023 - 128 * t2, channel_multiplier=-1)
            wins.append(w_t)
        # G[c] += sum_p window[p, c]: ones-lhsT matmuls, emitted AFTER
        # all Gram matmuls -- the Tensor engine runs its stream in
        # order, so interleaving these would stall Gram bank t2+1 on
        # window t2's skew/mask chain.
        for t2 in range(DT):
            for h in range(2):
                nc.tensor.matmul(
                    gps[h][:], ones[:],
                    wins[t2][:, h * 512:(h + 1) * 512],
                    start=(t2 == 0), stop=(t2 == DT - 1))
        g_sb = skp.tile([1, 1024], F32, tag='gsb')
        for h in range(2):
            nc.vector.tensor_copy(g_sb[:, h * 512:(h + 1) * 512], gps[h][:])
        nc.sync.dma_start(z_d[:, :], g_sb[:])

    nc.finalize()
    return nc


# ---------------------------------------------------------------- NEFF 2
def build_neff2():
    """out[l,d] = sum_m At[m,l] v[m,d] with At[m,l] = coef[(m-l) mod L].
    At is block-circulant: block (mt,lt) = D_{(mt-lt) mod 8} where
    D_j[k,m] = coef[(128j + k - m) mod 1024] -- only 8 distinct blocks,
    shipped as one [128, 1024] input. out tile lt = sum_j D_j @
    v[(lt+j) mod 8]; j-outer keeps the stationary D_j loaded for 8
    back-to-back matmuls with all 8 PSUM banks accumulating."""
    nc = bacc.Bacc(None, target_bir_lowering=False, debug=False)
    v_d = nc.declare_dram_parameter('v', [L, D], MM_DT, isOutput=False)
    d_d = nc.declare_dram_parameter('dall', [128, 1024], MM_DT, isOutput=False)
    o_d = nc.declare_dram_parameter('out', [128, 8 * D], F32, isOutput=True)

    LT = L // 128                      # 8

    with tile.TileContext(nc) as tc, ExitStack() as ctx:
        pool = ctx.enter_context(tc.tile_pool(name='sb', bufs=1))
        psum_o = ctx.enter_context(
            tc.tile_pool(name='pso', bufs=1, space=bass.MemorySpace.PSUM))

        d_sb = pool.tile([128, 1024], MM_DT)
        nc.scalar.dma_start(d_sb[:], d_d[:, :])
        # per-chunk v tiles so matmuls gate on 256KB arrivals
        v_ts = []
        for i in range(LT):
            v_t = pool.tile([128, D], MM_DT, tag=f'v{i}', name=f'v{i}')
            eng = nc.sync if i % 2 == 0 else nc.scalar
            eng.dma_start(v_t[:], v_d[i * 128:(i + 1) * 128, :])
            v_ts.append(v_t)

        pos = [psum_o.tile([128, D], F32, tag=f'po{lt}', name=f'po{lt}')
               for lt in range(LT)]
        o_sb = pool.tile([128, LT, D], F32)
        # phase A, m-outer over the first 4 v tiles: 8 matmuls per
        # arriving v tile (PE never starves); phase B, bank-outer over
        # the rest: banks complete staggered so PSUM copies and the two
        # output DMAs overlap the remaining matmuls.
        for m in range(4):
            for lt in range(LT):
                j = (m - lt) % LT
                nc.tensor.matmul(
                    pos[lt][:], d_sb[:, j * 128:(j + 1) * 128], v_ts[m][:],
                    start=(m == 0), stop=False)
        for lt in range(LT):
            for m in range(4, LT):
                j = (m - lt) % LT
                nc.tensor.matmul(
                    pos[lt][:], d_sb[:, j * 128:(j + 1) * 128], v_ts[m][:],
                    start=False, stop=(m == LT - 1))
            nc.vector.tensor_copy(o_sb[:, lt, :], pos[lt][:])
            if lt == 3:
                nc.sync.dma_start(o_d[:, 0:4 * D], o_sb[:, 0:4, :])
        # out row 128*lt + p lives at o_sb[p, lt, :]; host un-permutes
        nc.scalar.dma_start(o_d[:, 4 * D:], o_sb[:, 4:8, :])

    nc.finalize()
    return nc


# ---------------------------------------------------------------- driver
def _get_graphs():
    if 'nc1' not in _cache:
        _cache['nc1'] = build_neff1()
        _cache['nc2'] = build_neff2()
    return _cache['nc1'], _cache['nc2']


def kernel(queries, keys, values, _trace=False):
    tabs = _tables()
    nc1, nc2 = _get_graphs()
    q = np.ascontiguousarray(np.asarray(queries, np.float32))
    k = np.ascontiguousarray(np.asarray(keys, np.float32))
    v = np.ascontiguousarray(np.asarray(values, np.float32))

    in1 = [{'q': q[b], 'k': k[b]} for b in range(B)]
    r1 = run_bass_kernel_spmd(nc1, in1, core_ids=CORE_IDS, trace=_trace)
    # g[j] = diagonal sum of N at Delta = j - 512, computed on device
    g = np.stack([r1.results[b]['zout'].reshape(1024) for b in range(B)])
    mean_value = g.astype(np.float32) @ tabs['KER']           # [B, T]
    ind = np.argsort(-mean_value, axis=-1, kind='stable')[:, :K]
    val = np.take_along_axis(mean_value, ind, axis=-1)
    e = np.exp(val - val.max(-1, keepdims=True))
    w = e / e.sum(-1, keepdims=True)                          # [B, K]
    shifts = ind[0]                                           # [K]

    # 8 distinct circulant blocks: D_j[k, m] = coef[(128j + k - m) % L],
    # packed as dall[k, 128j + m]
    sh = shifts % L
    if 'didx' not in _cache:
        p_i = np.arange(128)[:, None, None]
        j_i = np.arange(8)[None, :, None]
        m_i = np.arange(128)[None, None, :]
        _cache['didx'] = ((128 * j_i + p_i - m_i) % L).reshape(128, 1024)
    didx = _cache['didx']
    in2 = []
    for b in range(B):
        coef = np.zeros(L, np.float32)
        np.add.at(coef, sh, w[b].astype(np.float32))
        in2.append({'v': v[b], 'dall': coef[didx]})
    r2 = run_bass_kernel_spmd(nc2, in2, core_ids=CORE_IDS, trace=_trace)
    out = np.stack([
        r2.results[b]['out'].reshape(128, 8, D)
        .transpose(1, 0, 2).reshape(L, D)
        for b in range(B)])                                   # [B, L, D]

    kernel._last_exec_ns = (
        (r1.exec_time_ns or 0) + (r2.exec_time_ns or 0)
        if (r1.exec_time_ns or r2.exec_time_ns) else None)
    kernel._last_results = (r1, r2)
    return out.astype(np.float32)
